# revision 20
# baseline (speedup 1.0000x reference)
"""Causal self-attention Trainium2 kernel (B=4, T=4096, C=384, H=6).

Sharding: 8 cores = 4 batches x 2 head-groups (3 heads each). Each core
computes y_partial = attn(x[b], heads hg) @ w_proj[rows of hg]; the host
sums the two partials per batch (the "all-reduce after c_proj" done on
host during unshard).

v2: chunk-granular softmax pipeline. The ACT engine (exp) is the
bottleneck engine (~1ns per free-element); everything is structured so it
never idles:
  - scores for each 128-key chunk land in a 2-bank PSUM ring slot,
    exp'd chunk-by-chunk into small per-chunk SBUF slots (so exp of tile
    N+1 overlaps att@v of tile N instead of serializing as before);
  - causal masking is pre-applied to the scores with a -1e9 triangle
    accumulated by an extra PE matmul (identity lhsT), so nothing sits
    between exp and att@v;
  - diagonal chunks only exp/att@v the live q-range (trapezoid);
  - att psums for the 3 heads accumulate in parallel banks per chunk;
  - phase A (qkv projection) of the next tile and c_proj of the previous
    tile are interleaved into the chunk loop as PE gap filler.
"""

import numpy as np
from contextlib import ExitStack

import concourse.bass as bass
import concourse.tile as tile
from concourse import mybir
from concourse.bass_utils import run_bass_kernel_spmd
from concourse.vector_clock import ScopedClock

F32 = mybir.dt.float32
BF16 = mybir.dt.bfloat16
EXP = mybir.ActivationFunctionType.Exp
MULT = mybir.AluOpType.mult

B, T, C, H, D = 4, 4096, 384, 6, 64
HPC = 3            # heads per core
QT = 512           # q tile
KC = 128           # key chunk
SCALE = 1.0 / 8.0  # 1/sqrt(64)
NEG = -1.0e9


# ---------------------------------------------------------------------------
# Workaround: neuronxcc CoreV3 rejects >2 sem waits on the Tile tail drain.
# Split the drain's waits into individual sync-engine wait instructions.
def _drain_and_barrier_split(self, tick_clock, wait_clock):
    nc = self.nc
    drain_inst = nc.sync.drain()
    wait_clock.add_sem_waits(
        drain_inst.ins, ScopedClock({None: tick_clock.global_clock})
    )
    si = drain_inst.ins.sync_info
    if si is not None and si.on_wait and len(si.on_wait) > 1:
        waits = list(si.on_wait)
        si.on_wait = []
        allocated = {h.name: h for h in self.sems.allocated().values()}
        for w in waits:
            h = allocated.get(w.ant_name)
            assert h is not None, f"no sem handle for drain wait {w.ant_name}"
            assert w.wait_mode == "sem-ge-imm", w.wait_mode
            nc.sync.wait_ge(h, w.wait_value)
    nc.all_engine_barrier()
    assert self.sems is not None
    popped = nc._tile_sem_poison_stack.pop()
    assert popped is self._sem_poison
    nc.clear_and_free_semaphores(list(self.sems.allocated().values()))
    nc.all_engine_barrier()


tile.TileContext._drain_and_barrier = _drain_and_barrier_split


MAX_WAITS = 1  # CoreV3 per-instruction sem-wait capacity (S3_LW holds only 1)


def _split_excess_waits(nc):
    """Hoist sem waits beyond MAX_WAITS onto same-engine NOPs inserted
    directly before the over-limit instruction (waits are order-free)."""
    for fn in nc.m.functions:
        for bb in fn.blocks:
            insts = list(bb.instructions)
            out = []
            changed = False
            for inst in insts:
                si = inst.sync_info
                if si is not None and si.on_wait and len(si.on_wait) > MAX_WAITS:
                    waits = list(si.on_wait)
                    excess, keep = waits[:-MAX_WAITS], waits[-MAX_WAITS:]
                    si.on_wait = keep
                    inst.sync_info = si
                    for i in range(0, len(excess), MAX_WAITS):
                        nop = mybir.InstNoOp(
                            name=f"{inst.name}-waitsplit-{i}", ins=[], outs=[]
                        )
                        nop.engine = inst.engine
                        nop.sync_info = mybir.SyncInfo(
                            on_wait=excess[i:i + MAX_WAITS], on_update=[]
                        )
                        nc.register_instruction(nop)
                        out.append(nop)
                    changed = True
                out.append(inst)
            if changed:
                bb.instructions = out


# ---------------------------------------------------------------------------


def build(t=T):
    nkc = t // KC          # key chunks total
    ntb = t // QT          # token blocks / q tiles

    nc = bass.Bass()
    x_d = nc.dram_tensor("xT16", [C, t], BF16, kind="ExternalInput")
    wq01_d = nc.dram_tensor("w_q01", [3, 128, 128], BF16, kind="ExternalInput")
    wk01_d = nc.dram_tensor("w_k01", [3, 128, 128], BF16, kind="ExternalInput")
    wqk2_d = nc.dram_tensor("w_qk2", [3, 128, 128], BF16, kind="ExternalInput")
    wv_d = nc.dram_tensor("w_v", [3, 128, 192], BF16, kind="ExternalInput")
    wo_d = nc.dram_tensor("w_o", [3, 64, 384], BF16, kind="ExternalInput")
    tri_d = nc.dram_tensor("trimask", [2, 128, 128], BF16, kind="ExternalInput")
    y_d = nc.dram_tensor("y", [t, C], F32, kind="ExternalOutput")

    with tile.TileContext(nc) as tc, ExitStack() as ctx:
        persist = ctx.enter_context(tc.tile_pool(name="persist", bufs=1))

        # weights / causal triangle mask
        wq01 = persist.tile([128, 3, 128], BF16)
        wk01 = persist.tile([128, 3, 128], BF16)
        wqk2 = persist.tile([128, 3, 128], BF16)
        wv = persist.tile([128, 3, 192], BF16)
        wo = persist.tile([64, 3, 384], BF16)
        trimask = persist.tile([128, 2, 128], BF16)
        for c in range(3):
            nc.sync.dma_start(out=wq01[:, c, :], in_=wq01_d[c])
            nc.sync.dma_start(out=wk01[:, c, :], in_=wk01_d[c])
            nc.sync.dma_start(out=wqk2[:, c, :], in_=wqk2_d[c])
            nc.sync.dma_start(out=wv[:, c, :], in_=wv_d[c])
            nc.sync.dma_start(out=wo[:, c, :], in_=wo_d[c])
        for j in range(2):
            nc.sync.dma_start(out=trimask[:, j, :], in_=tri_d[j])

        # persistent activations (bf16)
        qT01 = persist.tile([128, t], BF16)   # rows 0:64 h0 qT, 64:128 h1 qT
        kT01 = persist.tile([128, t], BF16)
        # head 2 q/k duplicated into both partition halves so chunk pairs
        # can run as concurrent row-group-packed matmuls
        qT2 = persist.tile([128, t], BF16)
        kT2 = persist.tile([128, t], BF16)
        vsb = persist.tile([128, nkc, 3, 65], BF16)  # [keys, chunk, head, d|one]
        nc.vector.memset(vsb[:, :, :, 64:65], 1.0)
        ones_col = persist.tile([1, 64], BF16)
        nc.vector.memset(ones_col[:], 1.0)

        with (
            tc.tile_pool(name="xt", bufs=3) as xt_p,
            tc.tile_pool(name="ps", bufs=2, space="PSUM") as ps_p,
            tc.tile_pool(name="ps_att", bufs=3, space="PSUM") as ps_att,
            tc.tile_pool(name="ps_x", bufs=1, space="PSUM") as ps_x,
            tc.tile_pool(name="pth01", bufs=8) as pth01_p,
            tc.tile_pool(name="pth2", bufs=5) as pth2_p,
            tc.tile_pool(name="attn", bufs=6) as attn_p,
            tc.tile_pool(name="lrow", bufs=3) as lrow_p,
            tc.tile_pool(name="lcol", bufs=6) as lcol_p,
            tc.tile_pool(name="yout", bufs=3) as yout_p,
        ):
            # -------------------------------------------------------------
            # phase A of block tb, split into callable pieces (interleaved
            # into the previous tile's chunk loop as PE gap filler)
            def phase_a_units(tb):
                units = []

                xT = xt_p.tile([128, 3, QT], BF16, tag="xt", name="xT")

                def u_load():
                    for c in range(3):
                        nc.sync.dma_start(
                            out=xT[:, c, :],
                            in_=x_d[c * 128:(c + 1) * 128,
                                    tb * QT:(tb + 1) * QT],
                        )
                units.append(u_load)

                def mk_qk(w_sb, dst01):
                    def u():
                        ps = ps_x.tile([128, QT], F32, tag="px", name="psqk")
                        for c in range(3):
                            nc.tensor.matmul(
                                ps[:, :], w_sb[:, c, :], xT[:, c, :],
                                start=(c == 0), stop=(c == 2),
                            )
                        if dst01 is not None:
                            nc.vector.tensor_copy(
                                dst01[:, tb * QT:(tb + 1) * QT], ps[:, :]
                            )
                        else:
                            # merged q2|k2: rows 0:64 -> qT2 low half,
                            # rows 64:128 -> kT2 high half (partition-aligned
                            # copies), then duplicate into the other halves.
                            nc.vector.tensor_copy(
                                qT2[0:64, tb * QT:(tb + 1) * QT],
                                ps[0:64, :],
                            )
                            nc.vector.tensor_copy(
                                kT2[64:128, tb * QT:(tb + 1) * QT],
                                ps[64:128, :],
                            )
                            nc.gpsimd.dma_start(
                                out=qT2[64:128, tb * QT:(tb + 1) * QT],
                                in_=qT2[0:64, tb * QT:(tb + 1) * QT],
                            )
                            nc.gpsimd.dma_start(
                                out=kT2[0:64, tb * QT:(tb + 1) * QT],
                                in_=kT2[64:128, tb * QT:(tb + 1) * QT],
                            )
                    return u

                units.append(mk_qk(wq01, qT01))
                units.append(mk_qk(wk01, kT01))
                units.append(mk_qk(wqk2, None))

                def u_v():
                    # all 4 token slices in one 2-bank psum tile so the 12
                    # accumulating matmuls run back-to-back (192 elems padded
                    # to 256 so no matmul output crosses a bank boundary)
                    psv = ps_p.tile([128, 4, 4, 64], F32, tag="ps", name="psv")
                    for s in range(4):
                        for c in range(3):
                            nc.tensor.matmul(
                                psv[:, s, 0:3, :].rearrange(
                                    "p h d -> p (h d)"),
                                xT[:, c, s * 128:(s + 1) * 128],
                                wv[:, c, :],
                                start=(c == 0), stop=(c == 2),
                            )
                    nc.vector.tensor_copy(
                        vsb[:, tb * 4:(tb + 1) * 4, :, 0:64],
                        psv[:, :, 0:3, :],
                    )
                units.append(u_v)
                return units

            # -------------------------------------------------------------
            # c_proj of tile pqt (attn tiles already normalized by 1/l):
            # the 3 heads accumulate in PSUM, one copy out per slice.
            def cproj_units(prev):
                pqt, p_attn = prev
                pq0 = pqt * QT
                units = []

                def mk(s):
                    def u():
                        ysb = yout_p.tile([128, C], F32, tag="ysb", name="ysb")
                        yp = ps_x.tile([128, C], F32, tag="px", name="yp")
                        for h in range(3):
                            nc.tensor.matmul(
                                yp[:],
                                p_attn[h][:, s * 128:(s + 1) * 128],
                                wo[:, h, :],
                                start=(h == 0), stop=(h == 2),
                            )
                        nc.vector.tensor_copy(ysb[:], yp[:])
                        nc.sync.dma_start(
                            out=y_d[pq0 + s * 128:pq0 + (s + 1) * 128, :],
                            in_=ysb[:],
                        )
                    return u
                for s in range(4):
                    units.append(mk(s))
                return units

            # -------------------------------------------------------------
            prev_tile = None   # (qt, attn_tiles, linv_tiles) awaiting cproj

            # phase A of block 0 runs up front (no earlier tile to hide in)
            for u in phase_a_units(0):
                u()

            for qt in range(ntb):
                nch = 4 * (qt + 1)
                q0, q1 = qt * QT, (qt + 1) * QT

                # filler work to interleave into this tile's chunk loop:
                # phase A of block qt+1, c_proj of tile qt-1
                filler = []
                if qt + 1 < ntb:
                    filler.extend(phase_a_units(qt + 1))
                if prev_tile is not None:
                    filler.extend(cproj_units(prev_tile))
                # spread filler over the chunk loop (denser early)
                fill_at = {}
                for i, u in enumerate(filler):
                    fill_at.setdefault(min(2 * i + 1, nch - 1), []).append(u)

                # per-head att psums, accumulated chunk-by-chunk
                att = [
                    ps_att.tile([65, QT], F32, tag="att", name=f"att{h}")
                    for h in range(3)
                ]

                pth01_tile = [None] * nch        # per-chunk tiles for h0/h1
                pth2_tile = [None] * (nch // 2)  # pair tiles for head 2

                def emit_scores(ck):
                    # scores h0/h1 (row-group packed) + -1e9 causal triangle
                    # + exp; head-2 scores ride on odd chunks as pairs.
                    j = ck - 4 * qt          # >=0 on the diagonal block
                    qlo = 128 * j if j >= 0 else 0
                    ssx = ps_p.tile([128, 2, QT], F32, tag="ps", name="ssx")
                    nc.tensor.matmul(
                        ssx[:, 0, qlo:],
                        kT01[0:64, ck * KC:(ck + 1) * KC],
                        qT01[0:64, q0 + qlo:q1],
                        start=True, stop=(j < 0), tile_position=(0, 0),
                    )
                    nc.tensor.matmul(
                        ssx[:, 1, qlo:],
                        kT01[64:128, ck * KC:(ck + 1) * KC],
                        qT01[64:128, q0 + qlo:q1],
                        start=True, stop=True, tile_position=(64, 0),
                    )
                    p01 = pth01_p.tile([128, 2, QT], BF16, tag="pth01",
                                       name="p01")
                    pth01_tile[ck] = (p01, qlo)
                    nc.scalar.activation(
                        out=p01[:, :, qlo:],
                        in_=ssx[:, :, qlo:],
                        func=EXP, scale=SCALE,
                    )
                    if j >= 0:
                        # zero the causal triangle of the diagonal 128-block
                        # (both heads in one op; off the exp critical path)
                        nc.vector.tensor_tensor(
                            out=p01[:, :, qlo:qlo + 128],
                            in0=p01[:, :, qlo:qlo + 128],
                            in1=trimask[:, :, :], op=MULT,
                        )

                    if ck % 2 == 1:
                        g = ck // 2
                        jp = 2 * g - 4 * qt      # j of the even chunk
                        qlo2 = 128 * jp if jp >= 0 else 0
                        ssc = ps_p.tile([128, 2, QT], F32, tag="ps",
                                        name="ssc")
                        nc.tensor.matmul(
                            ssc[:, 0, qlo2:],
                            kT2[0:64, (2 * g) * KC:(2 * g + 1) * KC],
                            qT2[0:64, q0 + qlo2:q1],
                            start=True, stop=(jp < 0), tile_position=(0, 0),
                        )
                        nc.tensor.matmul(
                            ssc[:, 1, qlo2:],
                            kT2[64:128, (2 * g + 1) * KC:(2 * g + 2) * KC],
                            qT2[64:128, q0 + qlo2:q1],
                            start=True, stop=True, tile_position=(64, 0),
                        )
                        p2 = pth2_p.tile([128, 2, QT], BF16, tag="pth2",
                                         name="p2")
                        pth2_tile[g] = (p2, qlo2)
                        nc.scalar.activation(
                            out=p2[:, :, qlo2:],
                            in_=ssc[:, :, qlo2:],
                            func=EXP, scale=SCALE,
                        )
                        if jp >= 0:
                            for half in range(2):
                                lo = qlo2 + 128 * half
                                nc.vector.tensor_tensor(
                                    out=p2[:, half, lo:lo + 128],
                                    in0=p2[:, half, lo:lo + 128],
                                    in1=trimask[:, 0, :], op=MULT,
                                )

                def emit_attv(ck):
                    p01, qlo = pth01_tile[ck]
                    for h in range(2):
                        nc.tensor.matmul(
                            att[h][:, qlo:], vsb[:, ck, h, :],
                            p01[:, h, qlo:],
                            start=(ck == 0), stop=(ck == nch - 1),
                        )
                    if ck % 2 == 1:
                        g = ck // 2
                        p2, qlo2 = pth2_tile[g]
                        for half in range(2):
                            cc = 2 * g + half
                            jj = cc - 4 * qt
                            ql = 128 * jj if jj >= 0 else 0
                            nc.tensor.matmul(
                                att[2][:, ql:], vsb[:, cc, 2, :],
                                p2[:, half, ql:],
                                start=(cc == 0), stop=(cc == nch - 1),
                            )

                # software-pipelined by one chunk: scores(ck+1) sits ahead
                # of att@v(ck) in the PE queue, so the PE stall on exp(ck)
                # never delays score production (ACT stays saturated).
                emit_scores(0)
                for ck in range(nch):
                    if ck + 1 < nch:
                        emit_scores(ck + 1)
                    emit_attv(ck)
                    for u in fill_at.get(ck, []):
                        u()

                # ---- tile tail: normalize attn by 1/l (row 64 of att) ----
                # broadcast 1/l across partitions with a contract-1 matmul
                # (ones[1,64]^T @ linv[1,512] -> psum), then scale the bf16
                # attn copy in place.
                attn_tiles = []
                for h in range(3):
                    linv = lrow_p.tile([1, QT], BF16, tag="linv", name="linv")
                    with nc.allow_low_precision(
                            reason="1/l broadcast rides a bf16 matmul"):
                        nc.vector.reciprocal(linv[:], att[h][64:65, :])
                    linvb = ps_x.tile([64, QT], F32, tag="px", name="linvb")
                    nc.tensor.matmul(
                        linvb[:], ones_col[:], linv[:], start=True, stop=True,
                    )
                    at = attn_p.tile([64, QT], BF16, tag="attn", name="at")
                    attn_tiles.append(at)
                    nc.vector.tensor_copy(at[:], att[h][0:64, :])
                    nc.vector.tensor_tensor(
                        out=at[:], in0=at[:], in1=linvb[:], op=MULT,
                    )

                prev_tile = (qt, attn_tiles)

            # epilogue: c_proj of the last tile
            for u in cproj_units(prev_tile):
                u()

    _split_excess_waits(nc)
    nc.finalize()
    return nc


_NC_CACHE = {}


def _get_nc(t=T):
    if t not in _NC_CACHE:
        _NC_CACHE[t] = build(t)
    return _NC_CACHE[t]


def _prep_core_inputs(x_b, w_attn, w_proj, hg, bf16):
    """Host-side shard prep for one core: batch x_b, head group hg (0/1)."""
    h0 = 3 * hg
    q = w_attn[:, 0:C]
    k = w_attn[:, C:2 * C]
    v = w_attn[:, 2 * C:3 * C]
    qcols = lambda h: q[:, h * D:(h + 1) * D]
    kcols = lambda h: k[:, h * D:(h + 1) * D]
    w_q01 = np.concatenate([qcols(h0), qcols(h0 + 1)], axis=1)      # [384,128]
    w_k01 = np.concatenate([kcols(h0), kcols(h0 + 1)], axis=1)
    w_qk2 = np.concatenate([qcols(h0 + 2), kcols(h0 + 2)], axis=1)  # [384,128]
    w_v = v[:, h0 * D:(h0 + 3) * D]                                 # [384,192]
    w_o = w_proj[h0 * D:(h0 + 3) * D, :]                            # [192,384]
    return {
        "xT16": np.ascontiguousarray(x_b.T, dtype=bf16),
        "w_q01": np.ascontiguousarray(w_q01.reshape(3, 128, 128), dtype=bf16),
        "w_k01": np.ascontiguousarray(w_k01.reshape(3, 128, 128), dtype=bf16),
        "w_qk2": np.ascontiguousarray(w_qk2.reshape(3, 128, 128), dtype=bf16),
        "w_v": np.ascontiguousarray(w_v.reshape(3, 128, 192), dtype=bf16),
        "w_o": np.ascontiguousarray(w_o.reshape(3, 64, 384), dtype=bf16),
    }


def _make_aux(bf16):
    p = np.arange(128)[:, None]
    f = np.arange(128)[None, :]
    tri = (f >= p).astype(np.float32).astype(bf16)
    return np.broadcast_to(tri, (2, 128, 128)).copy()


def kernel(x, w_attn, w_proj):
    import ml_dtypes
    bf16 = ml_dtypes.bfloat16

    x = np.asarray(x, dtype=np.float32)
    w_attn = np.asarray(w_attn, dtype=np.float32)
    w_proj = np.asarray(w_proj, dtype=np.float32)
    b, t, c = x.shape

    nc = _get_nc(t)
    trimask = _make_aux(bf16)
    in_maps = []
    for core in range(8):
        im = _prep_core_inputs(x[core // 2], w_attn, w_proj, core % 2, bf16)
        im["trimask"] = trimask
        in_maps.append(im)

    res = run_bass_kernel_spmd(nc, in_maps, list(range(8)))
    out = np.empty((b, t, c), dtype=np.float32)
    for bb in range(b):
        out[bb] = res.results[2 * bb]["y"] + res.results[2 * bb + 1]["y"]
    return out


# revision 23
# speedup vs baseline: 1.2256x; 1.2256x over previous
"""Causal self-attention Trainium2 kernel (B=4, T=4096, C=384, H=6).

Sharding: 8 cores = 4 batches x 2 head-groups (3 heads each). Each core
computes y_partial = attn(x[b], heads hg) @ w_proj[rows of hg]; the host
sums the two partials per batch (the "all-reduce after c_proj" done on
host during unshard).

v2: chunk-granular softmax pipeline. The ACT engine (exp) is the
bottleneck engine (~1ns per free-element); everything is structured so it
never idles:
  - scores for each 128-key chunk land in a 2-bank PSUM ring slot,
    exp'd chunk-by-chunk into small per-chunk SBUF slots (so exp of tile
    N+1 overlaps att@v of tile N instead of serializing as before);
  - causal masking is pre-applied to the scores with a -1e9 triangle
    accumulated by an extra PE matmul (identity lhsT), so nothing sits
    between exp and att@v;
  - diagonal chunks only exp/att@v the live q-range (trapezoid);
  - att psums for the 3 heads accumulate in parallel banks per chunk;
  - phase A (qkv projection) of the next tile and c_proj of the previous
    tile are interleaved into the chunk loop as PE gap filler.
"""

import numpy as np
from contextlib import ExitStack

import concourse.bass as bass
import concourse.tile as tile
from concourse import mybir
from concourse.bass_utils import run_bass_kernel_spmd
from concourse.vector_clock import ScopedClock

F32 = mybir.dt.float32
BF16 = mybir.dt.bfloat16
EXP = mybir.ActivationFunctionType.Exp
MULT = mybir.AluOpType.mult

B, T, C, H, D = 4, 4096, 384, 6, 64
HPC = 3            # heads per core
QT = 512           # q tile
KC = 128           # key chunk
SCALE = 1.0 / 8.0  # 1/sqrt(64)
NEG = -1.0e9


# ---------------------------------------------------------------------------
# Workaround: neuronxcc CoreV3 rejects >2 sem waits on the Tile tail drain.
# Split the drain's waits into individual sync-engine wait instructions.
def _drain_and_barrier_split(self, tick_clock, wait_clock):
    nc = self.nc
    drain_inst = nc.sync.drain()
    wait_clock.add_sem_waits(
        drain_inst.ins, ScopedClock({None: tick_clock.global_clock})
    )
    si = drain_inst.ins.sync_info
    if si is not None and si.on_wait and len(si.on_wait) > 1:
        waits = list(si.on_wait)
        si.on_wait = []
        allocated = {h.name: h for h in self.sems.allocated().values()}
        for w in waits:
            h = allocated.get(w.ant_name)
            assert h is not None, f"no sem handle for drain wait {w.ant_name}"
            assert w.wait_mode == "sem-ge-imm", w.wait_mode
            nc.sync.wait_ge(h, w.wait_value)
    nc.all_engine_barrier()
    assert self.sems is not None
    popped = nc._tile_sem_poison_stack.pop()
    assert popped is self._sem_poison
    nc.clear_and_free_semaphores(list(self.sems.allocated().values()))
    nc.all_engine_barrier()


tile.TileContext._drain_and_barrier = _drain_and_barrier_split


MAX_WAITS = 1  # CoreV3 per-instruction sem-wait capacity (S3_LW holds only 1)


def _split_excess_waits(nc):
    """Hoist sem waits beyond MAX_WAITS onto same-engine NOPs inserted
    directly before the over-limit instruction (waits are order-free)."""
    for fn in nc.m.functions:
        for bb in fn.blocks:
            insts = list(bb.instructions)
            out = []
            changed = False
            for inst in insts:
                si = inst.sync_info
                if si is not None and si.on_wait and len(si.on_wait) > MAX_WAITS:
                    waits = list(si.on_wait)
                    excess, keep = waits[:-MAX_WAITS], waits[-MAX_WAITS:]
                    si.on_wait = keep
                    inst.sync_info = si
                    for i in range(0, len(excess), MAX_WAITS):
                        nop = mybir.InstNoOp(
                            name=f"{inst.name}-waitsplit-{i}", ins=[], outs=[]
                        )
                        nop.engine = inst.engine
                        nop.sync_info = mybir.SyncInfo(
                            on_wait=excess[i:i + MAX_WAITS], on_update=[]
                        )
                        nc.register_instruction(nop)
                        out.append(nop)
                    changed = True
                out.append(inst)
            if changed:
                bb.instructions = out


# ---------------------------------------------------------------------------


def build(t=T):
    nkc = t // KC          # key chunks total
    ntb = t // QT          # token blocks / q tiles

    nc = bass.Bass()
    x_d = nc.dram_tensor("xT16", [C, t], BF16, kind="ExternalInput")
    wq01_d = nc.dram_tensor("w_q01", [3, 128, 128], BF16, kind="ExternalInput")
    wk01_d = nc.dram_tensor("w_k01", [3, 128, 128], BF16, kind="ExternalInput")
    wqk2_d = nc.dram_tensor("w_qk2", [3, 128, 128], BF16, kind="ExternalInput")
    wv_d = nc.dram_tensor("w_v", [3, 128, 192], BF16, kind="ExternalInput")
    wo_d = nc.dram_tensor("w_o", [3, 64, 384], BF16, kind="ExternalInput")
    tri_d = nc.dram_tensor("trimask", [2, 128, 128], BF16, kind="ExternalInput")
    y_d = nc.dram_tensor("y", [t, C], F32, kind="ExternalOutput")
    # scratch for transposing the softmax denominator row into columns
    l_d = nc.dram_tensor("lscratch", [t // QT, 3, QT], F32)

    with tile.TileContext(nc) as tc, ExitStack() as ctx:
        persist = ctx.enter_context(tc.tile_pool(name="persist", bufs=1))

        # weights / causal triangle mask
        wq01 = persist.tile([128, 3, 128], BF16)
        wk01 = persist.tile([128, 3, 128], BF16)
        wqk2 = persist.tile([128, 3, 128], BF16)
        wv = persist.tile([128, 3, 192], BF16)
        wo = persist.tile([64, 3, 384], BF16)
        trimask = persist.tile([128, 2, 128], BF16)
        for c in range(3):
            nc.sync.dma_start(out=wq01[:, c, :], in_=wq01_d[c])
            nc.sync.dma_start(out=wk01[:, c, :], in_=wk01_d[c])
            nc.sync.dma_start(out=wqk2[:, c, :], in_=wqk2_d[c])
            nc.sync.dma_start(out=wv[:, c, :], in_=wv_d[c])
            nc.sync.dma_start(out=wo[:, c, :], in_=wo_d[c])
        for j in range(2):
            nc.sync.dma_start(out=trimask[:, j, :], in_=tri_d[j])

        # persistent activations (bf16)
        qT01 = persist.tile([128, t], BF16)   # rows 0:64 h0 qT, 64:128 h1 qT
        kT01 = persist.tile([128, t], BF16)
        # head 2 q/k duplicated into both partition halves so chunk pairs
        # can run as concurrent row-group-packed matmuls
        qT2 = persist.tile([128, t], BF16)
        kT2 = persist.tile([128, t], BF16)
        vsb = persist.tile([128, nkc, 3, 65], BF16)  # [keys, chunk, head, d|one]
        nc.vector.memset(vsb[:, :, :, 64:65], 1.0)

        with (
            tc.tile_pool(name="xt", bufs=3) as xt_p,
            tc.tile_pool(name="ps", bufs=2, space="PSUM") as ps_p,
            tc.tile_pool(name="ps_att", bufs=3, space="PSUM") as ps_att,
            tc.tile_pool(name="ps_x", bufs=1, space="PSUM") as ps_x,
            tc.tile_pool(name="pth01", bufs=8) as pth01_p,
            tc.tile_pool(name="pth2", bufs=5) as pth2_p,
            tc.tile_pool(name="attn", bufs=6) as attn_p,
            tc.tile_pool(name="lrow", bufs=3) as lrow_p,
            tc.tile_pool(name="lcol", bufs=6) as lcol_p,
            tc.tile_pool(name="yout", bufs=3) as yout_p,
        ):
            # -------------------------------------------------------------
            # phase A of block tb, split into callable pieces (interleaved
            # into the previous tile's chunk loop as PE gap filler)
            def phase_a_units(tb):
                units = []

                xT = xt_p.tile([128, 3, QT], BF16, tag="xt", name="xT")

                def u_load():
                    for c in range(3):
                        nc.sync.dma_start(
                            out=xT[:, c, :],
                            in_=x_d[c * 128:(c + 1) * 128,
                                    tb * QT:(tb + 1) * QT],
                        )
                units.append(u_load)

                def mk_qk(w_sb, dst01):
                    def u():
                        ps = ps_x.tile([128, QT], F32, tag="px", name="psqk")
                        for c in range(3):
                            nc.tensor.matmul(
                                ps[:, :], w_sb[:, c, :], xT[:, c, :],
                                start=(c == 0), stop=(c == 2),
                            )
                        if dst01 is not None:
                            nc.vector.tensor_copy(
                                dst01[:, tb * QT:(tb + 1) * QT], ps[:, :]
                            )
                        else:
                            # merged q2|k2: rows 0:64 -> qT2 low half,
                            # rows 64:128 -> kT2 high half (partition-aligned
                            # copies), then duplicate into the other halves.
                            nc.vector.tensor_copy(
                                qT2[0:64, tb * QT:(tb + 1) * QT],
                                ps[0:64, :],
                            )
                            nc.vector.tensor_copy(
                                kT2[64:128, tb * QT:(tb + 1) * QT],
                                ps[64:128, :],
                            )
                            nc.gpsimd.dma_start(
                                out=qT2[64:128, tb * QT:(tb + 1) * QT],
                                in_=qT2[0:64, tb * QT:(tb + 1) * QT],
                            )
                            nc.gpsimd.dma_start(
                                out=kT2[0:64, tb * QT:(tb + 1) * QT],
                                in_=kT2[64:128, tb * QT:(tb + 1) * QT],
                            )
                    return u

                units.append(mk_qk(wq01, qT01))
                units.append(mk_qk(wk01, kT01))
                units.append(mk_qk(wqk2, None))

                def u_v():
                    # all 4 token slices in one 2-bank psum tile so the 12
                    # accumulating matmuls run back-to-back (192 elems padded
                    # to 256 so no matmul output crosses a bank boundary)
                    psv = ps_p.tile([128, 4, 4, 64], F32, tag="ps", name="psv")
                    for s in range(4):
                        for c in range(3):
                            nc.tensor.matmul(
                                psv[:, s, 0:3, :].rearrange(
                                    "p h d -> p (h d)"),
                                xT[:, c, s * 128:(s + 1) * 128],
                                wv[:, c, :],
                                start=(c == 0), stop=(c == 2),
                            )
                    nc.vector.tensor_copy(
                        vsb[:, tb * 4:(tb + 1) * 4, :, 0:64],
                        psv[:, :, 0:3, :],
                    )
                units.append(u_v)
                return units

            # -------------------------------------------------------------
            # c_proj of tile pqt, per-head 1/l applied as per-partition
            # scalars on the q-partition-major psum slices
            def cproj_units(prev):
                pqt, p_attn, p_linv = prev
                pq0 = pqt * QT
                units = []

                def mk(s):
                    def u():
                        ysb = yout_p.tile([128, C], F32, tag="ysb", name="ysb")
                        for h in range(3):
                            yp = ps_x.tile([128, C], F32, tag="px", name="yp")
                            nc.tensor.matmul(
                                yp[:],
                                p_attn[h][:, s * 128:(s + 1) * 128],
                                wo[:, h, :],
                                start=True, stop=True,
                            )
                            sc = p_linv[h][:, s:s + 1]
                            if h == 0:
                                nc.vector.tensor_scalar(
                                    out=ysb[:], in0=yp[:], scalar1=sc,
                                    scalar2=None, op0=MULT,
                                )
                            else:
                                nc.vector.scalar_tensor_tensor(
                                    out=ysb[:], in0=yp[:], scalar=sc,
                                    in1=ysb[:],
                                    op0=MULT, op1=mybir.AluOpType.add,
                                )
                        nc.sync.dma_start(
                            out=y_d[pq0 + s * 128:pq0 + (s + 1) * 128, :],
                            in_=ysb[:],
                        )
                    return u
                for s in range(4):
                    units.append(mk(s))
                return units

            # -------------------------------------------------------------
            prev_tile = None   # (qt, attn_tiles, linv_tiles) awaiting cproj

            # phase A of block 0 runs up front (no earlier tile to hide in)
            for u in phase_a_units(0):
                u()

            state = {}

            def tile_state(qt):
                if qt not in state:
                    state[qt] = {
                        "nch": 4 * (qt + 1),
                        "att": [ps_att.tile([65, QT], F32, tag="att",
                                            name=f"att{h}") for h in range(3)],
                        "p01": {}, "p2": {},
                        "fill_at": None,
                    }
                return state[qt]

            def make_fill(qt):
                # computed at the first att@v of tile qt, i.e. after
                # emit_tail(qt-1) has published prev_tile
                nch = 4 * (qt + 1)
                filler = []
                if qt + 1 < ntb:
                    filler.extend(phase_a_units(qt + 1))
                if prev_tile[0] is not None:
                    filler.extend(cproj_units(prev_tile[0]))
                    prev_tile[0] = None
                fill_at = {}
                for i, u in enumerate(filler):
                    fill_at.setdefault(
                        min(2 * i + 1, max(nch - 2, 1)), []).append(u)
                return fill_at

            def emit_scores(qt, ck):
                st = tile_state(qt)
                q0, q1 = qt * QT, (qt + 1) * QT
                j = ck - 4 * qt
                qlo = 128 * j if j >= 0 else 0
                ssx = ps_p.tile([128, 2, QT], F32, tag="ps", name="ssx")
                nc.tensor.matmul(
                    ssx[:, 0, qlo:],
                    kT01[0:64, ck * KC:(ck + 1) * KC],
                    qT01[0:64, q0 + qlo:q1],
                    start=True, stop=True, tile_position=(0, 0),
                )
                nc.tensor.matmul(
                    ssx[:, 1, qlo:],
                    kT01[64:128, ck * KC:(ck + 1) * KC],
                    qT01[64:128, q0 + qlo:q1],
                    start=True, stop=True, tile_position=(64, 0),
                )
                p01 = pth01_p.tile([128, 2, QT], BF16, tag="pth01", name="p01")
                st["p01"][ck] = (p01, qlo)
                nc.scalar.activation(
                    out=p01[:, :, qlo:], in_=ssx[:, :, qlo:],
                    func=EXP, scale=SCALE,
                )
                if j >= 0:
                    nc.vector.tensor_tensor(
                        out=p01[:, :, qlo:qlo + 128],
                        in0=p01[:, :, qlo:qlo + 128],
                        in1=trimask[:, :, :], op=MULT,
                    )

                if ck % 2 == 1:
                    g = ck // 2
                    jp = 2 * g - 4 * qt
                    qlo2 = 128 * jp if jp >= 0 else 0
                    ssc = ps_p.tile([128, 2, QT], F32, tag="ps", name="ssc")
                    nc.tensor.matmul(
                        ssc[:, 0, qlo2:],
                        kT2[0:64, (2 * g) * KC:(2 * g + 1) * KC],
                        qT2[0:64, q0 + qlo2:q1],
                        start=True, stop=True, tile_position=(0, 0),
                    )
                    nc.tensor.matmul(
                        ssc[:, 1, qlo2:],
                        kT2[64:128, (2 * g + 1) * KC:(2 * g + 2) * KC],
                        qT2[64:128, q0 + qlo2:q1],
                        start=True, stop=True, tile_position=(64, 0),
                    )
                    p2 = pth2_p.tile([128, 2, QT], BF16, tag="pth2", name="p2")
                    st["p2"][g] = (p2, qlo2)
                    nc.scalar.activation(
                        out=p2[:, :, qlo2:], in_=ssc[:, :, qlo2:],
                        func=EXP, scale=SCALE,
                    )
                    if jp >= 0:
                        for half in range(2):
                            lo = qlo2 + 128 * half
                            nc.vector.tensor_tensor(
                                out=p2[:, half, lo:lo + 128],
                                in0=p2[:, half, lo:lo + 128],
                                in1=trimask[:, 0, :], op=MULT,
                            )

            def emit_attv(qt, ck):
                st = tile_state(qt)
                if st["fill_at"] is None:
                    st["fill_at"] = make_fill(qt)
                nch = st["nch"]
                att = st["att"]
                p01, qlo = st["p01"].pop(ck)
                for h in range(2):
                    nc.tensor.matmul(
                        att[h][:, qlo:], vsb[:, ck, h, :],
                        p01[:, h, qlo:],
                        start=(ck == 0), stop=(ck == nch - 1),
                    )
                if ck % 2 == 1:
                    g = ck // 2
                    p2, qlo2 = st["p2"].pop(g)
                    for half in range(2):
                        cc = 2 * g + half
                        jj = cc - 4 * qt
                        ql = 128 * jj if jj >= 0 else 0
                        nc.tensor.matmul(
                            att[2][:, ql:], vsb[:, cc, 2, :],
                            p2[:, half, ql:],
                            start=(cc == 0), stop=(cc == nch - 1),
                        )

            def emit_tail(qt):
                # copy attn to sbuf; transpose the denominator row into
                # per-partition columns via a DRAM roundtrip, reciprocal
                st = tile_state(qt)
                att = st["att"]
                attn_tiles = []
                linv_tiles = []
                for h in range(3):
                    at = attn_p.tile([64, QT], BF16, tag="attn", name="at")
                    attn_tiles.append(at)
                    nc.vector.tensor_copy(at[:], att[h][0:64, :])
                    lrow = lrow_p.tile([65, QT], F32, tag="lrow", name="lrow")
                    nc.vector.tensor_copy(lrow[64:65, :], att[h][64:65, :])
                    nc.sync.dma_start(out=l_d[qt, h], in_=lrow[64:65, :])
                    lcol = lcol_p.tile([128, 4], F32, tag="lcol", name="lcol")
                    nc.sync.dma_start(
                        out=lcol[:],
                        in_=l_d[qt, h].rearrange("(s p) -> p s", p=128),
                    )
                    linv = lcol_p.tile([128, 4], F32, tag="linv", name="linv")
                    linv_tiles.append(linv)
                    nc.vector.reciprocal(linv[:], lcol[:])
                prev_tile[0] = (qt, attn_tiles, linv_tiles)

            prev_tile = [None]
            seq = [(qt, ck) for qt in range(ntb)
                   for ck in range(4 * (qt + 1))]
            emit_scores(*seq[0])
            for i, (qt, ck) in enumerate(seq):
                if i + 1 < len(seq):
                    emit_scores(*seq[i + 1])
                emit_attv(qt, ck)
                if ck == state[qt]["nch"] - 1:
                    emit_tail(qt)
                for u in state[qt]["fill_at"].get(ck, []):
                    u()

            # epilogue: c_proj of the last tile
            for u in cproj_units(prev_tile[0]):
                u()

    _split_excess_waits(nc)
    nc.finalize()
    return nc


_NC_CACHE = {}


def _get_nc(t=T):
    if t not in _NC_CACHE:
        _NC_CACHE[t] = build(t)
    return _NC_CACHE[t]


def _prep_core_inputs(x_b, w_attn, w_proj, hg, bf16):
    """Host-side shard prep for one core: batch x_b, head group hg (0/1)."""
    h0 = 3 * hg
    q = w_attn[:, 0:C]
    k = w_attn[:, C:2 * C]
    v = w_attn[:, 2 * C:3 * C]
    qcols = lambda h: q[:, h * D:(h + 1) * D]
    kcols = lambda h: k[:, h * D:(h + 1) * D]
    w_q01 = np.concatenate([qcols(h0), qcols(h0 + 1)], axis=1)      # [384,128]
    w_k01 = np.concatenate([kcols(h0), kcols(h0 + 1)], axis=1)
    w_qk2 = np.concatenate([qcols(h0 + 2), kcols(h0 + 2)], axis=1)  # [384,128]
    w_v = v[:, h0 * D:(h0 + 3) * D]                                 # [384,192]
    w_o = w_proj[h0 * D:(h0 + 3) * D, :]                            # [192,384]
    return {
        "xT16": np.ascontiguousarray(x_b.T, dtype=bf16),
        "w_q01": np.ascontiguousarray(w_q01.reshape(3, 128, 128), dtype=bf16),
        "w_k01": np.ascontiguousarray(w_k01.reshape(3, 128, 128), dtype=bf16),
        "w_qk2": np.ascontiguousarray(w_qk2.reshape(3, 128, 128), dtype=bf16),
        "w_v": np.ascontiguousarray(w_v.reshape(3, 128, 192), dtype=bf16),
        "w_o": np.ascontiguousarray(w_o.reshape(3, 64, 384), dtype=bf16),
    }


def _make_aux(bf16):
    p = np.arange(128)[:, None]
    f = np.arange(128)[None, :]
    tri = (f >= p).astype(np.float32).astype(bf16)
    return np.broadcast_to(tri, (2, 128, 128)).copy()


def kernel(x, w_attn, w_proj):
    import ml_dtypes
    bf16 = ml_dtypes.bfloat16

    x = np.asarray(x, dtype=np.float32)
    w_attn = np.asarray(w_attn, dtype=np.float32)
    w_proj = np.asarray(w_proj, dtype=np.float32)
    b, t, c = x.shape

    nc = _get_nc(t)
    trimask = _make_aux(bf16)
    in_maps = []
    for core in range(8):
        im = _prep_core_inputs(x[core // 2], w_attn, w_proj, core % 2, bf16)
        im["trimask"] = trimask
        in_maps.append(im)

    res = run_bass_kernel_spmd(nc, in_maps, list(range(8)))
    out = np.empty((b, t, c), dtype=np.float32)
    for bb in range(b):
        out[bb] = res.results[2 * bb]["y"] + res.results[2 * bb + 1]["y"]
    return out


# revision 25
# speedup vs baseline: 1.2788x; 1.0435x over previous
"""Causal self-attention Trainium2 kernel (B=4, T=4096, C=384, H=6).

Sharding: 8 cores = 4 batches x 2 head-groups (3 heads each). Each core
computes y_partial = attn(x[b], heads hg) @ w_proj[rows of hg]; the host
sums the two partials per batch (the "all-reduce after c_proj" done on
host during unshard).

v2: chunk-granular softmax pipeline. The ACT engine (exp) is the
bottleneck engine (~1ns per free-element); everything is structured so it
never idles:
  - scores for each 128-key chunk land in a 2-bank PSUM ring slot,
    exp'd chunk-by-chunk into small per-chunk SBUF slots (so exp of tile
    N+1 overlaps att@v of tile N instead of serializing as before);
  - causal masking is pre-applied to the scores with a -1e9 triangle
    accumulated by an extra PE matmul (identity lhsT), so nothing sits
    between exp and att@v;
  - diagonal chunks only exp/att@v the live q-range (trapezoid);
  - att psums for the 3 heads accumulate in parallel banks per chunk;
  - phase A (qkv projection) of the next tile and c_proj of the previous
    tile are interleaved into the chunk loop as PE gap filler.
"""

import numpy as np
from contextlib import ExitStack

import concourse.bass as bass
import concourse.tile as tile
from concourse import mybir
from concourse.bass_utils import run_bass_kernel_spmd
from concourse.vector_clock import ScopedClock

F32 = mybir.dt.float32
BF16 = mybir.dt.bfloat16
EXP = mybir.ActivationFunctionType.Exp
MULT = mybir.AluOpType.mult

B, T, C, H, D = 4, 4096, 384, 6, 64
HPC = 3            # heads per core
QT = 512           # q tile
KC = 128           # key chunk
SCALE = 1.0 / 8.0  # 1/sqrt(64)
NEG = -1.0e9


# ---------------------------------------------------------------------------
# Workaround: neuronxcc CoreV3 rejects >2 sem waits on the Tile tail drain.
# Split the drain's waits into individual sync-engine wait instructions.
def _drain_and_barrier_split(self, tick_clock, wait_clock):
    nc = self.nc
    drain_inst = nc.sync.drain()
    wait_clock.add_sem_waits(
        drain_inst.ins, ScopedClock({None: tick_clock.global_clock})
    )
    si = drain_inst.ins.sync_info
    if si is not None and si.on_wait and len(si.on_wait) > 1:
        waits = list(si.on_wait)
        si.on_wait = []
        allocated = {h.name: h for h in self.sems.allocated().values()}
        for w in waits:
            h = allocated.get(w.ant_name)
            assert h is not None, f"no sem handle for drain wait {w.ant_name}"
            assert w.wait_mode == "sem-ge-imm", w.wait_mode
            nc.sync.wait_ge(h, w.wait_value)
    nc.all_engine_barrier()
    assert self.sems is not None
    popped = nc._tile_sem_poison_stack.pop()
    assert popped is self._sem_poison
    nc.clear_and_free_semaphores(list(self.sems.allocated().values()))
    nc.all_engine_barrier()


tile.TileContext._drain_and_barrier = _drain_and_barrier_split


MAX_WAITS = 1  # CoreV3 per-instruction sem-wait capacity (S3_LW holds only 1)


def _split_excess_waits(nc):
    """Hoist sem waits beyond MAX_WAITS onto same-engine NOPs inserted
    directly before the over-limit instruction (waits are order-free)."""
    for fn in nc.m.functions:
        for bb in fn.blocks:
            insts = list(bb.instructions)
            out = []
            changed = False
            for inst in insts:
                si = inst.sync_info
                if si is not None and si.on_wait and len(si.on_wait) > MAX_WAITS:
                    waits = list(si.on_wait)
                    excess, keep = waits[:-MAX_WAITS], waits[-MAX_WAITS:]
                    si.on_wait = keep
                    inst.sync_info = si
                    for i in range(0, len(excess), MAX_WAITS):
                        nop = mybir.InstNoOp(
                            name=f"{inst.name}-waitsplit-{i}", ins=[], outs=[]
                        )
                        nop.engine = inst.engine
                        nop.sync_info = mybir.SyncInfo(
                            on_wait=excess[i:i + MAX_WAITS], on_update=[]
                        )
                        nc.register_instruction(nop)
                        out.append(nop)
                    changed = True
                out.append(inst)
            if changed:
                bb.instructions = out


# ---------------------------------------------------------------------------


def build(t=T):
    nkc = t // KC          # key chunks total
    ntb = t // QT          # token blocks / q tiles

    nc = bass.Bass()
    x_d = nc.dram_tensor("xT16", [C, t], BF16, kind="ExternalInput")
    wq01_d = nc.dram_tensor("w_q01", [3, 128, 128], BF16, kind="ExternalInput")
    wk01_d = nc.dram_tensor("w_k01", [3, 128, 128], BF16, kind="ExternalInput")
    wqk2_d = nc.dram_tensor("w_qk2", [3, 128, 128], BF16, kind="ExternalInput")
    wv_d = nc.dram_tensor("w_v", [3, 128, 192], BF16, kind="ExternalInput")
    wo_d = nc.dram_tensor("w_o", [3, 64, 384], BF16, kind="ExternalInput")
    tri_d = nc.dram_tensor("trimask", [2, 128, 128], BF16, kind="ExternalInput")
    y_d = nc.dram_tensor("y", [t, C], F32, kind="ExternalOutput")
    # scratch for transposing the softmax denominator row into columns
    l_d = nc.dram_tensor("lscratch", [t // QT, 3, QT], F32)

    with tile.TileContext(nc) as tc, ExitStack() as ctx:
        persist = ctx.enter_context(tc.tile_pool(name="persist", bufs=1))

        # weights / causal triangle mask
        wq01 = persist.tile([128, 3, 128], BF16)
        wk01 = persist.tile([128, 3, 128], BF16)
        wqk2 = persist.tile([128, 3, 128], BF16)
        wv = persist.tile([128, 3, 192], BF16)
        wo = persist.tile([64, 3, 384], BF16)
        trimask = persist.tile([128, 2, 128], BF16)
        qs = [nc.sync, nc.scalar, nc.gpsimd]
        qi = 0
        for c in range(3):
            for dst, srcd in ((wq01, wq01_d), (wk01, wk01_d),
                              (wqk2, wqk2_d), (wv, wv_d), (wo, wo_d)):
                qs[qi % len(qs)].dma_start(out=dst[:, c, :], in_=srcd[c])
                qi += 1
        for j in range(2):
            qs[qi % len(qs)].dma_start(out=trimask[:, j, :], in_=tri_d[j])
            qi += 1

        # persistent activations (bf16)
        qT01 = persist.tile([128, t], BF16)   # rows 0:64 h0 qT, 64:128 h1 qT
        kT01 = persist.tile([128, t], BF16)
        # head 2 q/k duplicated into both partition halves so chunk pairs
        # can run as concurrent row-group-packed matmuls
        qT2 = persist.tile([128, t], BF16)
        kT2 = persist.tile([128, t], BF16)
        vsb = persist.tile([128, nkc, 3, 65], BF16)  # [keys, chunk, head, d|one]
        nc.vector.memset(vsb[:, :, :, 64:65], 1.0)

        with (
            tc.tile_pool(name="xt", bufs=3) as xt_p,
            tc.tile_pool(name="ps", bufs=2, space="PSUM") as ps_p,
            tc.tile_pool(name="ps_att", bufs=3, space="PSUM") as ps_att,
            tc.tile_pool(name="ps_x", bufs=1, space="PSUM") as ps_x,
            tc.tile_pool(name="pth01", bufs=8) as pth01_p,
            tc.tile_pool(name="pth2", bufs=5) as pth2_p,
            tc.tile_pool(name="attn", bufs=6) as attn_p,
            tc.tile_pool(name="lrow", bufs=3) as lrow_p,
            tc.tile_pool(name="lcol", bufs=6) as lcol_p,
            tc.tile_pool(name="yout", bufs=3) as yout_p,
        ):
            # -------------------------------------------------------------
            # phase A of block tb, split into callable pieces (interleaved
            # into the previous tile's chunk loop as PE gap filler)
            def phase_a_units(tb):
                units = []

                xT = xt_p.tile([128, 3, QT], BF16, tag="xt", name="xT")

                def u_load():
                    for c in range(3):
                        nc.sync.dma_start(
                            out=xT[:, c, :],
                            in_=x_d[c * 128:(c + 1) * 128,
                                    tb * QT:(tb + 1) * QT],
                        )
                units.append(u_load)

                def mk_qk(w_sb, dst01):
                    def u():
                        ps = ps_x.tile([128, QT], F32, tag="px", name="psqk")
                        for c in range(3):
                            nc.tensor.matmul(
                                ps[:, :], w_sb[:, c, :], xT[:, c, :],
                                start=(c == 0), stop=(c == 2),
                            )
                        if dst01 is not None:
                            nc.vector.tensor_copy(
                                dst01[:, tb * QT:(tb + 1) * QT], ps[:, :]
                            )
                        else:
                            # merged q2|k2: rows 0:64 -> qT2 low half,
                            # rows 64:128 -> kT2 high half (partition-aligned
                            # copies), then duplicate into the other halves.
                            nc.vector.tensor_copy(
                                qT2[0:64, tb * QT:(tb + 1) * QT],
                                ps[0:64, :],
                            )
                            nc.vector.tensor_copy(
                                kT2[64:128, tb * QT:(tb + 1) * QT],
                                ps[64:128, :],
                            )
                            nc.gpsimd.dma_start(
                                out=qT2[64:128, tb * QT:(tb + 1) * QT],
                                in_=qT2[0:64, tb * QT:(tb + 1) * QT],
                            )
                            nc.gpsimd.dma_start(
                                out=kT2[0:64, tb * QT:(tb + 1) * QT],
                                in_=kT2[64:128, tb * QT:(tb + 1) * QT],
                            )
                    return u

                units.append(mk_qk(wq01, qT01))
                units.append(mk_qk(wk01, kT01))
                units.append(mk_qk(wqk2, None))

                def mk_v(s0):
                    # two token slices per 1-bank psum tile (192 elems padded
                    # to 256 so no matmul output crosses a bank boundary)
                    def u():
                        psv = ps_x.tile([128, 2, 4, 64], F32, tag="px",
                                        name="psv")
                        for s in range(2):
                            for c in range(3):
                                nc.tensor.matmul(
                                    psv[:, s, 0:3, :].rearrange(
                                        "p h d -> p (h d)"),
                                    xT[:, c,
                                       (s0 + s) * 128:(s0 + s + 1) * 128],
                                    wv[:, c, :],
                                    start=(c == 0), stop=(c == 2),
                                )
                        nc.vector.tensor_copy(
                            vsb[:, tb * 4 + s0:tb * 4 + s0 + 2, :, 0:64],
                            psv[:, :, 0:3, :],
                        )
                    return u
                units.append(mk_v(0))
                units.append(mk_v(2))
                return units

            # -------------------------------------------------------------
            # c_proj of tile pqt, per-head 1/l applied as per-partition
            # scalars on the q-partition-major psum slices
            def cproj_units(prev, pool=None):
                pqt, p_attn, p_linv = prev
                pq0 = pqt * QT
                ypool, ytag = (pool, "ps") if pool is not None else (ps_x, "px")
                units = []

                def mk(s):
                    def u():
                        ysb = yout_p.tile([128, C], F32, tag="ysb", name="ysb")
                        for h in range(3):
                            yp = ypool.tile([128, C], F32, tag=ytag, name="yp")
                            nc.tensor.matmul(
                                yp[:],
                                p_attn[h][:, s * 128:(s + 1) * 128],
                                wo[:, h, :],
                                start=True, stop=True,
                            )
                            sc = p_linv[h][:, s:s + 1]
                            if h == 0:
                                nc.vector.tensor_scalar(
                                    out=ysb[:], in0=yp[:], scalar1=sc,
                                    scalar2=None, op0=MULT,
                                )
                            else:
                                nc.vector.scalar_tensor_tensor(
                                    out=ysb[:], in0=yp[:], scalar=sc,
                                    in1=ysb[:],
                                    op0=MULT, op1=mybir.AluOpType.add,
                                )
                        nc.sync.dma_start(
                            out=y_d[pq0 + s * 128:pq0 + (s + 1) * 128, :],
                            in_=ysb[:],
                        )
                    return u
                for s in range(4):
                    units.append(mk(s))
                return units

            # -------------------------------------------------------------
            prev_tile = None   # (qt, attn_tiles, linv_tiles) awaiting cproj

            # phase A of block 0 runs up front (no earlier tile to hide in)
            for u in phase_a_units(0):
                u()

            state = {}

            def tile_state(qt):
                if qt not in state:
                    state[qt] = {
                        "nch": 4 * (qt + 1),
                        "att": [ps_att.tile([65, QT], F32, tag="att",
                                            name=f"att{h}") for h in range(3)],
                        "p01": {}, "p2": {},
                        "fill_at": None,
                    }
                return state[qt]

            def make_fill(qt):
                # computed at the first att@v of tile qt, i.e. after
                # emit_tail(qt-1) has published prev_tile
                nch = 4 * (qt + 1)
                filler = []
                if qt + 1 < ntb:
                    filler.extend(phase_a_units(qt + 1))
                if prev_tile[0] is not None:
                    filler.extend(cproj_units(prev_tile[0]))
                    prev_tile[0] = None
                fill_at = {}
                for i, u in enumerate(filler):
                    fill_at.setdefault(
                        min(1 + i, max(nch - 2, 1)), []).append(u)
                return fill_at

            def emit_scores(qt, ck):
                st = tile_state(qt)
                q0, q1 = qt * QT, (qt + 1) * QT
                j = ck - 4 * qt
                qlo = 128 * j if j >= 0 else 0
                ssx = ps_p.tile([128, 2, QT], F32, tag="ps", name="ssx")
                nc.tensor.matmul(
                    ssx[:, 0, qlo:],
                    kT01[0:64, ck * KC:(ck + 1) * KC],
                    qT01[0:64, q0 + qlo:q1],
                    start=True, stop=True, tile_position=(0, 0),
                )
                nc.tensor.matmul(
                    ssx[:, 1, qlo:],
                    kT01[64:128, ck * KC:(ck + 1) * KC],
                    qT01[64:128, q0 + qlo:q1],
                    start=True, stop=True, tile_position=(64, 0),
                )
                p01 = pth01_p.tile([128, 2, QT], BF16, tag="pth01", name="p01")
                st["p01"][ck] = (p01, qlo)
                nc.scalar.activation(
                    out=p01[:, :, qlo:], in_=ssx[:, :, qlo:],
                    func=EXP, scale=SCALE,
                )
                if j >= 0:
                    nc.vector.tensor_tensor(
                        out=p01[:, :, qlo:qlo + 128],
                        in0=p01[:, :, qlo:qlo + 128],
                        in1=trimask[:, :, :], op=MULT,
                    )

                if ck % 2 == 1:
                    g = ck // 2
                    jp = 2 * g - 4 * qt
                    qlo2 = 128 * jp if jp >= 0 else 0
                    ssc = ps_p.tile([128, 2, QT], F32, tag="ps", name="ssc")
                    nc.tensor.matmul(
                        ssc[:, 0, qlo2:],
                        kT2[0:64, (2 * g) * KC:(2 * g + 1) * KC],
                        qT2[0:64, q0 + qlo2:q1],
                        start=True, stop=True, tile_position=(0, 0),
                    )
                    nc.tensor.matmul(
                        ssc[:, 1, qlo2:],
                        kT2[64:128, (2 * g + 1) * KC:(2 * g + 2) * KC],
                        qT2[64:128, q0 + qlo2:q1],
                        start=True, stop=True, tile_position=(64, 0),
                    )
                    p2 = pth2_p.tile([128, 2, QT], BF16, tag="pth2", name="p2")
                    st["p2"][g] = (p2, qlo2)
                    nc.scalar.activation(
                        out=p2[:, :, qlo2:], in_=ssc[:, :, qlo2:],
                        func=EXP, scale=SCALE,
                    )
                    if jp >= 0:
                        for half in range(2):
                            lo = qlo2 + 128 * half
                            nc.vector.tensor_tensor(
                                out=p2[:, half, lo:lo + 128],
                                in0=p2[:, half, lo:lo + 128],
                                in1=trimask[:, 0, :], op=MULT,
                            )

            def emit_attv(qt, ck):
                st = tile_state(qt)
                if st["fill_at"] is None:
                    st["fill_at"] = make_fill(qt)
                nch = st["nch"]
                att = st["att"]
                p01, qlo = st["p01"].pop(ck)
                for h in range(2):
                    nc.tensor.matmul(
                        att[h][:, qlo:], vsb[:, ck, h, :],
                        p01[:, h, qlo:],
                        start=(ck == 0), stop=(ck == nch - 1),
                    )
                if ck % 2 == 1:
                    g = ck // 2
                    p2, qlo2 = st["p2"].pop(g)
                    for half in range(2):
                        cc = 2 * g + half
                        jj = cc - 4 * qt
                        ql = 128 * jj if jj >= 0 else 0
                        nc.tensor.matmul(
                            att[2][:, ql:], vsb[:, cc, 2, :],
                            p2[:, half, ql:],
                            start=(cc == 0), stop=(cc == nch - 1),
                        )

            def emit_tail(qt):
                # copy attn to sbuf; transpose the denominator row into
                # per-partition columns via a DRAM roundtrip, reciprocal
                st = tile_state(qt)
                att = st["att"]
                attn_tiles = []
                linv_tiles = []
                for h in range(3):
                    at = attn_p.tile([64, QT], BF16, tag="attn", name="at")
                    attn_tiles.append(at)
                    nc.vector.tensor_copy(at[:], att[h][0:64, :])
                    lrow = lrow_p.tile([65, QT], F32, tag="lrow", name="lrow")
                    nc.vector.tensor_copy(lrow[64:65, :], att[h][64:65, :])
                    nc.sync.dma_start(out=l_d[qt, h], in_=lrow[64:65, :])
                    lcol = lcol_p.tile([128, 4], F32, tag="lcol", name="lcol")
                    nc.sync.dma_start(
                        out=lcol[:],
                        in_=l_d[qt, h].rearrange("(s p) -> p s", p=128),
                    )
                    linv = lcol_p.tile([128, 4], F32, tag="linv", name="linv")
                    linv_tiles.append(linv)
                    nc.vector.reciprocal(linv[:], lcol[:])
                prev_tile[0] = (qt, attn_tiles, linv_tiles)

            prev_tile = [None]
            seq = [(qt, ck) for qt in range(ntb)
                   for ck in range(4 * (qt + 1))]
            emit_scores(*seq[0])
            for i, (qt, ck) in enumerate(seq):
                if i + 1 < len(seq):
                    emit_scores(*seq[i + 1])
                emit_attv(qt, ck)
                if ck == state[qt]["nch"] - 1:
                    emit_tail(qt)
                for u in state[qt]["fill_at"].get(ck, []):
                    u()

            # epilogue: c_proj of the last tile (scores ring is idle now)
            for u in cproj_units(prev_tile[0], pool=ps_p):
                u()

    _split_excess_waits(nc)
    nc.finalize()
    return nc


_NC_CACHE = {}


def _get_nc(t=T):
    if t not in _NC_CACHE:
        _NC_CACHE[t] = build(t)
    return _NC_CACHE[t]


def _prep_core_inputs(x_b, w_attn, w_proj, hg, bf16):
    """Host-side shard prep for one core: batch x_b, head group hg (0/1)."""
    h0 = 3 * hg
    q = w_attn[:, 0:C]
    k = w_attn[:, C:2 * C]
    v = w_attn[:, 2 * C:3 * C]
    qcols = lambda h: q[:, h * D:(h + 1) * D]
    kcols = lambda h: k[:, h * D:(h + 1) * D]
    w_q01 = np.concatenate([qcols(h0), qcols(h0 + 1)], axis=1)      # [384,128]
    w_k01 = np.concatenate([kcols(h0), kcols(h0 + 1)], axis=1)
    w_qk2 = np.concatenate([qcols(h0 + 2), kcols(h0 + 2)], axis=1)  # [384,128]
    w_v = v[:, h0 * D:(h0 + 3) * D]                                 # [384,192]
    w_o = w_proj[h0 * D:(h0 + 3) * D, :]                            # [192,384]
    return {
        "xT16": np.ascontiguousarray(x_b.T, dtype=bf16),
        "w_q01": np.ascontiguousarray(w_q01.reshape(3, 128, 128), dtype=bf16),
        "w_k01": np.ascontiguousarray(w_k01.reshape(3, 128, 128), dtype=bf16),
        "w_qk2": np.ascontiguousarray(w_qk2.reshape(3, 128, 128), dtype=bf16),
        "w_v": np.ascontiguousarray(w_v.reshape(3, 128, 192), dtype=bf16),
        "w_o": np.ascontiguousarray(w_o.reshape(3, 64, 384), dtype=bf16),
    }


def _make_aux(bf16):
    p = np.arange(128)[:, None]
    f = np.arange(128)[None, :]
    tri = (f >= p).astype(np.float32).astype(bf16)
    return np.broadcast_to(tri, (2, 128, 128)).copy()


def kernel(x, w_attn, w_proj):
    import ml_dtypes
    bf16 = ml_dtypes.bfloat16

    x = np.asarray(x, dtype=np.float32)
    w_attn = np.asarray(w_attn, dtype=np.float32)
    w_proj = np.asarray(w_proj, dtype=np.float32)
    b, t, c = x.shape

    nc = _get_nc(t)
    trimask = _make_aux(bf16)
    in_maps = []
    for core in range(8):
        im = _prep_core_inputs(x[core // 2], w_attn, w_proj, core % 2, bf16)
        im["trimask"] = trimask
        in_maps.append(im)

    res = run_bass_kernel_spmd(nc, in_maps, list(range(8)))
    out = np.empty((b, t, c), dtype=np.float32)
    for bb in range(b):
        out[bb] = res.results[2 * bb]["y"] + res.results[2 * bb + 1]["y"]
    return out
